# revision 1
# baseline (speedup 1.0000x reference)
"""HEALPix padding (p=2) kernel for Trainium2 (Bass/Tile).

Input : data (96, 256, 64, 64) f32 = (B*12 faces, C, H, W), B=8, plus scalar p=2.
Output: (96, 256, 68, 68) f32.

Sharding: data-parallel over the batch dim. Each of the 8 NeuronCores gets one
group of 12 HEALPix faces (12, 256, 64, 64) so every cross-face halo gather is
core-local.

Per-core plan (per 128-channel chunk, channels on SBUF partitions):
  - Two staging DMAs bring the first-2 / last-2 rows of every face into SBUF.
  - Face tiles stream through SBUF (contiguous 2MB loads). On arrival the
    tile's first-2/last-2 columns are extracted on-chip (column strips are
    non-contiguous in DRAM, so DMAing them directly would be descriptor-bound)
    and its interior is copied into the padded 68x68 plane.
  - Once a face's neighbor column strips are all staged, its halo strips and
    corners are assembled from the staged edges and the finished plane is
    stored with one contiguous 2.3MB DMA.
The face load order is chosen so the column-strip producer of each face lands
before its consumers while keeping at most ~5 padded planes live.
"""

import numpy as np

_FACES = 12
_PAD = 2

# Load order: keeps peak live planes ~5 while satisfying column-strip deps.
_ORDER = [1, 2, 6, 0, 5, 3, 7, 4, 9, 10, 11, 8]


def _col_deps(g):
    """Faces whose column strips face g's halo assembly reads."""
    if g < 4:  # _pn
        return ((g + 1) % 4, 4 + (g + 1) % 4)
    if g < 8:  # _pe
        i = g - 4
        return (i, (i + 3) % 4, 8 + i)
    i = g - 8  # _ps
    return (4 + i, 8 + (i + 3) % 4)


def _assemble(nc, g, pl, colL, colR, toprows, botrows, H, OH):
    """Emit halo strip + corner ops for face g into plane `pl`.

    colL[f]/colR[f]: (P, H, 2) staged first/last-2 columns of face f.
    toprows/botrows: (P, 12, 2, W) staged first/last-2 rows of all faces.
    """
    W = H
    V = nc.vector
    p2 = pl.rearrange("p a b -> p (a b)")
    tr_f = toprows.rearrange("p f r w -> p (f r w)")
    br_f = botrows.rearrange("p f r w -> p (f r w)")

    if g < 4:  # _pn
        i = g
        t = (i + 1) % 4
        tl = (i + 2) % 4
        l = (i + 3) % 4
        bl = l
        b = 4 + i
        br = 8 + i
        r = 4 + (i + 1) % 4
        tr = t
        # top[r_, c] = t[c, 1 - r_]   (rot90 of t's first-2 cols)
        for r_ in range(2):
            V.tensor_copy(pl[:, r_, 2:2 + W], colL[t][:, :, 1 - r_])
        # left[i_, j] = l[1 - j, i_]  (rot90 of l's first-2 rows)
        for j in range(2):
            V.tensor_copy(pl[:, 2:2 + H, j], toprows[:, l, 1 - j, :])
        V.tensor_copy(pl[:, H + 2:H + 4, 2:2 + W], toprows[:, b, :, :])
        V.tensor_copy(pl[:, 2:2 + H, W + 2:W + 4], colL[r][:])
        # tl corner = rot180(tl_face[0:2, 0:2])
        for i_ in range(2):
            for j_ in range(2):
                V.tensor_copy(pl[:, i_:i_ + 1, j_:j_ + 1],
                       toprows[:, tl, 1 - i_:2 - i_, 1 - j_:2 - j_])
        V.tensor_copy(pl[:, H + 2:H + 4, 0:2], toprows[:, bl, :, W - 2:W])
        V.tensor_copy(pl[:, 0:2, W + 2:W + 4], botrows[:, tr, :, 0:2])
        V.tensor_copy(pl[:, H + 2:H + 4, W + 2:W + 4], toprows[:, br, :, 0:2])

    elif g < 8:  # _pe
        i = g - 4
        t = i
        l = (i + 3) % 4
        bl = 4 + (i + 3) % 4
        b = 8 + (i + 3) % 4
        r = 8 + i
        tr = 4 + (i + 1) % 4
        V.tensor_copy(pl[:, 0:2, 2:2 + W], botrows[:, t, :, :])
        V.tensor_copy(pl[:, 2:2 + H, 0:2], colR[l][:])
        V.tensor_copy(pl[:, H + 2:H + 4, 2:2 + W], toprows[:, b, :, :])
        V.tensor_copy(pl[:, 2:2 + H, W + 2:W + 4], colL[r][:])
        # tl corner (computed): [[.5(t[H-2,0]+l[0,W-2]), t[H-2,0]],
        #                        [l[0,W-2], .5(t[H-1,0]+l[0,W-1])]]
        V.tensor_copy(pl[:, 0:1, 1:2], colL[t][:, H - 2:H - 1, 0:1])
        V.tensor_copy(pl[:, 1:2, 0:1], toprows[:, l, 0:1, W - 2:W - 1])
        d = p2[:, 0:OH + 2:OH + 1]
        V.tensor_add(d, colL[t].rearrange("p a b -> p (a b)")[:, 2 * (H - 2):2 * H:2],
                     tr_f[:, l * 2 * W + W - 2:l * 2 * W + W])
        V.tensor_scalar_mul(d, d, 0.5)
        # br corner (computed): [[.5(b[0,W-1]+r[H-1,0]), r[H-1,1]],
        #                        [b[1,W-1], .5(b[1,W-1]+r[H-1,1])]]
        V.tensor_copy(pl[:, H + 2:H + 3, W + 3:W + 4], botrows[:, r, 1:2, 1:2])
        V.tensor_copy(pl[:, H + 3:H + 4, W + 2:W + 3], toprows[:, b, 1:2, W - 1:W])
        st = (H + 2) * OH + (W + 2)
        d = p2[:, st:st + OH + 2:OH + 1]
        V.tensor_add(d, tr_f[:, b * 2 * W + W - 1:b * 2 * W + 2 * W:W],
                     br_f[:, r * 2 * W + W:r * 2 * W + W + 2])
        V.tensor_scalar_mul(d, d, 0.5)
        V.tensor_copy(pl[:, H + 2:H + 4, 0:2], toprows[:, bl, :, W - 2:W])
        V.tensor_copy(pl[:, 0:2, W + 2:W + 4], botrows[:, tr, :, 0:2])

    else:  # _ps
        i = g - 8
        t = 4 + (i + 1) % 4
        tl = i
        l = 4 + i
        bl = 8 + (i + 3) % 4
        b = bl
        br = 8 + (i + 2) % 4
        r = 8 + (i + 1) % 4
        tr = r
        V.tensor_copy(pl[:, 0:2, 2:2 + W], botrows[:, t, :, :])
        V.tensor_copy(pl[:, 2:2 + H, 0:2], colR[l][:])
        # bottom[r_, c] = b[c, W-1-r_]  (rot90 of b's last-2 cols)
        for r_ in range(2):
            V.tensor_copy(pl[:, H + 2 + r_, 2:2 + W], colR[b][:, :, 1 - r_])
        # right[i_, j] = r[H-1-j, i_]   (rot90 of r's last-2 rows)
        for j in range(2):
            V.tensor_copy(pl[:, 2:2 + H, W + 2 + j], botrows[:, r, 1 - j, :])
        V.tensor_copy(pl[:, 0:2, 0:2], botrows[:, tl, :, W - 2:W])
        V.tensor_copy(pl[:, H + 2:H + 4, 0:2], toprows[:, bl, :, W - 2:W])
        V.tensor_copy(pl[:, 0:2, W + 2:W + 4], botrows[:, tr, :, 0:2])
        # br corner = rot180(br_face[H-2:H, W-2:W])
        for i_ in range(2):
            for j_ in range(2):
                V.tensor_copy(pl[:, H + 2 + i_:H + 3 + i_, W + 2 + j_:W + 3 + j_],
                       botrows[:, br, 1 - i_:2 - i_, W - 1 - j_:W - j_])


def _build_nc(C=256, H=64, PCHUNK=128, tiles_bufs=2, planes_bufs=6):
    import concourse.bass as bass
    import concourse.mybir as mybir
    import concourse.tile_scheduler as _ts
    import concourse.tile_sem_assignment as _tsa
    from concourse.tile import TileContext

    # All HWDGE DMAs here issue from the SP engine (one FIFO ring), so one
    # completion-tracking lane is both sufficient and tighter: with 8
    # round-robin lanes, slot-reuse deps span two DMAHW sems and the DMA
    # instruction exceeds walrus's sync-wait slot limit ("Too many sync wait
    # commands" in CoreV2Gen setupSyncWait).
    _ts.NUM_HWDGE_SEMS = 1
    _tsa.NUM_HWDGE_SEMS = 1

    f32 = mybir.dt.float32
    W = H
    OH = H + 2 * _PAD
    nc = bass.Bass()
    x = nc.dram_tensor("data", (_FACES, C, H, W), f32, kind="ExternalInput")
    y = nc.dram_tensor("out", (_FACES, C, OH, OH), f32, kind="ExternalOutput")

    with TileContext(nc) as tc:
        with (
            tc.tile_pool(name="tiles", bufs=tiles_bufs) as tpool,
            tc.tile_pool(name="planes", bufs=planes_bufs) as ppool,
            tc.tile_pool(name="rows", bufs=4) as rpool,
            tc.tile_pool(name="cols", bufs=26) as cpool,
        ):
            for c0 in range(0, C, PCHUNK):
                P = PCHUNK
                cs = slice(c0, c0 + P)
                toprows = rpool.tile([P, _FACES, 2, W], f32,
                                     name=f"toprows_{c0}", tag="rows")
                botrows = rpool.tile([P, _FACES, 2, W], f32,
                                     name=f"botrows_{c0}", tag="rows")
                nc.sync.dma_start(out=toprows[:],
                                    in_=x[:, cs, 0:2, :].transpose((1, 0, 2, 3)))
                nc.sync.dma_start(out=botrows[:],
                                    in_=x[:, cs, H - 2:H, :].transpose((1, 0, 2, 3)))

                colL, colR, planes = {}, {}, {}
                loaded, assembled = set(), set()
                for f in _ORDER:
                    tile = tpool.tile([P, H, W], f32,
                                      name=f"tile_{c0}_{f}", tag="tile")
                    nc.sync.dma_start(
                        out=tile.rearrange("p a b -> p (a b)"),
                        in_=x[f, cs].rearrange("c a b -> c (a b)"))
                    cl = cpool.tile([P, H, 2], f32, name=f"colL_{c0}_{f}", tag="col")
                    cr = cpool.tile([P, H, 2], f32, name=f"colR_{c0}_{f}", tag="col")
                    nc.vector.tensor_copy(cl[:], tile[:, :, 0:2])
                    nc.vector.tensor_copy(cr[:], tile[:, :, W - 2:W])
                    colL[f], colR[f] = cl, cr
                    pl = ppool.tile([P, OH, OH], f32,
                                    name=f"plane_{c0}_{f}", tag="plane")
                    nc.vector.tensor_copy(pl[:, 2:2 + H, 2:2 + W], tile[:])
                    planes[f] = pl
                    loaded.add(f)
                    for g in _ORDER:
                        if g in assembled or g not in loaded:
                            continue
                        if all(d in loaded for d in _col_deps(g)):
                            _assemble(nc, g, planes[g], colL, colR,
                                      toprows, botrows, H, OH)
                            nc.sync.dma_start(
                                out=y[g, cs].rearrange("c a b -> c (a b)"),
                                in_=planes[g].rearrange("p a b -> p (a b)"))
                            assembled.add(g)
                assert len(assembled) == _FACES

    # walrus's DMA_DIRECT2D lowering accepts a single sync-wait slot, but
    # slot-reuse deps give some DMAs two (compute sem + DMAHW sem). Every DMA
    # here issues from the SP sequencer in program order onto one HWDGE ring
    # (qSPDynamicHW), and per-ring full-completion order equals issue order,
    # so DMA-vs-DMA semaphore waits are redundant: drop them, keeping the
    # compute-engine wait.
    import concourse.mybir as mybir
    max_dve_wait_on_dma = 0
    for blk in nc.m.functions[0].blocks:
        for inst in blk.instructions:
            if not isinstance(inst, mybir.InstDMACopy):
                continue
            assert inst.engine == mybir.EngineType.SP, inst.concise()
            si = inst.sync_info
            if si is None:
                continue
            for w in si.on_wait:
                if w.ant_name.startswith("DVE"):
                    max_dve_wait_on_dma = max(max_dve_wait_on_dma, w.wait_value)
            if len(si.on_wait) <= 1:
                continue
            keep = [w for w in si.on_wait if not w.ant_name.startswith("DMAHW")]
            if not keep:
                keep = [max(si.on_wait, key=lambda w: w.wait_value)]
            assert len(keep) == 1, [w.ant_name for w in si.on_wait]
            si.on_wait = keep
            inst.sync_info = si

    # The SP kernel-tail Drain waits on [DVE_total, DMAHW0_total]; the final
    # store DMA already waits on the same DVE total and the DMAHW0 wait
    # covers that store's completion, so the DVE wait is transitively
    # implied — drop it to fit the 1-wait slot.
    for blk in nc.m.functions[0].blocks:
        for inst in blk.instructions:
            si = inst.sync_info
            if si is None or len(si.on_wait) <= 1:
                continue
            assert isinstance(inst, mybir.InstDrain), inst.concise()
            dve = [w for w in si.on_wait if w.ant_name.startswith("DVE")]
            dma = [w for w in si.on_wait if w.ant_name.startswith("DMAHW")]
            assert len(dve) == 1 and len(dma) == 1, inst.concise()
            assert dve[0].wait_value <= max_dve_wait_on_dma, inst.concise()
            si.on_wait = dma
            inst.sync_info = si

    nc.finalize()
    return nc


_NC_CACHE = {}


def _get_nc():
    if "nc" not in _NC_CACHE:
        _NC_CACHE["nc"] = _build_nc()
    return _NC_CACHE["nc"]


def _run(data, **kwargs):
    from concourse import bass_utils

    data = np.ascontiguousarray(np.asarray(data, dtype=np.float32))
    n_cores = 8
    group = data.shape[0] // n_cores
    assert group == _FACES
    nc = _get_nc()
    in_maps = [{"data": data[g * group:(g + 1) * group]} for g in range(n_cores)]
    return bass_utils.run_bass_kernel_spmd(
        nc, in_maps, core_ids=list(range(n_cores)), **kwargs)


def kernel(data, p):
    assert int(p) == _PAD
    res = _run(data)
    return np.concatenate([r["out"] for r in res.results], axis=0)



# revision 3
# speedup vs baseline: 1.1754x; 1.1754x over previous
"""HEALPix padding (p=2) kernel for Trainium2 (Bass/Tile).

Input : data (96, 256, 64, 64) f32 = (B*12 faces, C, H, W), B=8, plus scalar p=2.
Output: (96, 256, 68, 68) f32.

Sharding: data-parallel over the batch dim. Each of the 8 NeuronCores gets one
group of 12 HEALPix faces (12, 256, 64, 64) so every cross-face halo gather is
core-local.

Per-core plan (per 128-channel chunk, channels on SBUF partitions):
  - Two staging DMAs bring the first-2 / last-2 rows of every face into SBUF.
  - Face tiles stream through SBUF (contiguous 2MB loads). On arrival the
    tile's first-2/last-2 columns are extracted on-chip (column strips are
    non-contiguous in DRAM, so DMAing them directly would be descriptor-bound)
    and its interior is copied into the padded 68x68 plane.
  - Once a face's neighbor column strips are all staged, its halo strips and
    corners are assembled from the staged edges and the finished plane is
    stored with one contiguous 2.3MB DMA.
The face load order is chosen so the column-strip producer of each face lands
before its consumers while keeping at most ~5 padded planes live.
"""

import numpy as np

_FACES = 12
_PAD = 2

# Load order: keeps peak live planes ~5 while satisfying column-strip deps.
_ORDER = [1, 2, 6, 0, 5, 3, 7, 4, 9, 10, 11, 8]


def _col_deps(g):
    """Faces whose column strips face g's halo assembly reads."""
    if g < 4:  # _pn
        return ((g + 1) % 4, 4 + (g + 1) % 4)
    if g < 8:  # _pe
        i = g - 4
        return (i, (i + 3) % 4, 8 + i)
    i = g - 8  # _ps
    return (4 + i, 8 + (i + 3) % 4)


def _assemble(nc, g, pl, colL, colR, toprows, botrows, H, OH):
    """Emit halo strip + corner ops for face g into plane `pl`.

    colL[f]/colR[f]: (P, H, 2) staged first/last-2 columns of face f.
    toprows/botrows: (P, 12, 2, W) staged first/last-2 rows of all faces.
    """
    W = H
    V = nc.vector
    p2 = pl.rearrange("p a b -> p (a b)")
    tr_f = toprows.rearrange("p f r w -> p (f r w)")
    br_f = botrows.rearrange("p f r w -> p (f r w)")

    if g < 4:  # _pn
        i = g
        t = (i + 1) % 4
        tl = (i + 2) % 4
        l = (i + 3) % 4
        bl = l
        b = 4 + i
        br = 8 + i
        r = 4 + (i + 1) % 4
        tr = t
        # top[r_, c] = t[c, 1 - r_]   (rot90 of t's first-2 cols)
        for r_ in range(2):
            V.tensor_copy(pl[:, r_, 2:2 + W], colL[t][:, :, 1 - r_])
        # left[i_, j] = l[1 - j, i_]  (rot90 of l's first-2 rows)
        for j in range(2):
            V.tensor_copy(pl[:, 2:2 + H, j], toprows[:, l, 1 - j, :])
        V.tensor_copy(pl[:, H + 2:H + 4, 2:2 + W], toprows[:, b, :, :])
        V.tensor_copy(pl[:, 2:2 + H, W + 2:W + 4], colL[r][:])
        # tl corner = rot180(tl_face[0:2, 0:2])
        for i_ in range(2):
            for j_ in range(2):
                V.tensor_copy(pl[:, i_:i_ + 1, j_:j_ + 1],
                       toprows[:, tl, 1 - i_:2 - i_, 1 - j_:2 - j_])
        V.tensor_copy(pl[:, H + 2:H + 4, 0:2], toprows[:, bl, :, W - 2:W])
        V.tensor_copy(pl[:, 0:2, W + 2:W + 4], botrows[:, tr, :, 0:2])
        V.tensor_copy(pl[:, H + 2:H + 4, W + 2:W + 4], toprows[:, br, :, 0:2])

    elif g < 8:  # _pe
        i = g - 4
        t = i
        l = (i + 3) % 4
        bl = 4 + (i + 3) % 4
        b = 8 + (i + 3) % 4
        r = 8 + i
        tr = 4 + (i + 1) % 4
        V.tensor_copy(pl[:, 0:2, 2:2 + W], botrows[:, t, :, :])
        V.tensor_copy(pl[:, 2:2 + H, 0:2], colR[l][:])
        V.tensor_copy(pl[:, H + 2:H + 4, 2:2 + W], toprows[:, b, :, :])
        V.tensor_copy(pl[:, 2:2 + H, W + 2:W + 4], colL[r][:])
        # tl corner (computed): [[.5(t[H-2,0]+l[0,W-2]), t[H-2,0]],
        #                        [l[0,W-2], .5(t[H-1,0]+l[0,W-1])]]
        V.tensor_copy(pl[:, 0:1, 1:2], colL[t][:, H - 2:H - 1, 0:1])
        V.tensor_copy(pl[:, 1:2, 0:1], toprows[:, l, 0:1, W - 2:W - 1])
        d = p2[:, 0:OH + 2:OH + 1]
        V.tensor_add(d, colL[t].rearrange("p a b -> p (a b)")[:, 2 * (H - 2):2 * H:2],
                     tr_f[:, l * 2 * W + W - 2:l * 2 * W + W])
        V.tensor_scalar_mul(d, d, 0.5)
        # br corner (computed): [[.5(b[0,W-1]+r[H-1,0]), r[H-1,1]],
        #                        [b[1,W-1], .5(b[1,W-1]+r[H-1,1])]]
        V.tensor_copy(pl[:, H + 2:H + 3, W + 3:W + 4], botrows[:, r, 1:2, 1:2])
        V.tensor_copy(pl[:, H + 3:H + 4, W + 2:W + 3], toprows[:, b, 1:2, W - 1:W])
        st = (H + 2) * OH + (W + 2)
        d = p2[:, st:st + OH + 2:OH + 1]
        V.tensor_add(d, tr_f[:, b * 2 * W + W - 1:b * 2 * W + 2 * W:W],
                     br_f[:, r * 2 * W + W:r * 2 * W + W + 2])
        V.tensor_scalar_mul(d, d, 0.5)
        V.tensor_copy(pl[:, H + 2:H + 4, 0:2], toprows[:, bl, :, W - 2:W])
        V.tensor_copy(pl[:, 0:2, W + 2:W + 4], botrows[:, tr, :, 0:2])

    else:  # _ps
        i = g - 8
        t = 4 + (i + 1) % 4
        tl = i
        l = 4 + i
        bl = 8 + (i + 3) % 4
        b = bl
        br = 8 + (i + 2) % 4
        r = 8 + (i + 1) % 4
        tr = r
        V.tensor_copy(pl[:, 0:2, 2:2 + W], botrows[:, t, :, :])
        V.tensor_copy(pl[:, 2:2 + H, 0:2], colR[l][:])
        # bottom[r_, c] = b[c, W-1-r_]  (rot90 of b's last-2 cols)
        for r_ in range(2):
            V.tensor_copy(pl[:, H + 2 + r_, 2:2 + W], colR[b][:, :, 1 - r_])
        # right[i_, j] = r[H-1-j, i_]   (rot90 of r's last-2 rows)
        for j in range(2):
            V.tensor_copy(pl[:, 2:2 + H, W + 2 + j], botrows[:, r, 1 - j, :])
        V.tensor_copy(pl[:, 0:2, 0:2], botrows[:, tl, :, W - 2:W])
        V.tensor_copy(pl[:, H + 2:H + 4, 0:2], toprows[:, bl, :, W - 2:W])
        V.tensor_copy(pl[:, 0:2, W + 2:W + 4], botrows[:, tr, :, 0:2])
        # br corner = rot180(br_face[H-2:H, W-2:W])
        for i_ in range(2):
            for j_ in range(2):
                V.tensor_copy(pl[:, H + 2 + i_:H + 3 + i_, W + 2 + j_:W + 3 + j_],
                       botrows[:, br, 1 - i_:2 - i_, W - 1 - j_:W - j_])


def _build_nc(C=256, H=64, PCHUNK=128, tiles_bufs=2, planes_bufs=6):
    import concourse.bass as bass
    import concourse.mybir as mybir
    import concourse.tile_scheduler as _ts
    import concourse.tile_sem_assignment as _tsa
    from concourse.tile import TileContext

    # Two completion-tracking lanes: with one lane the framework chains every
    # DMA on its predecessor's full completion (lane-tick reuse), costing a
    # 2175ns sem-prop+DGE bubble per DMA. With two lanes the chain wait is
    # "DMA k-2 complete" — already satisfied in steady state — and real data
    # hazards still get compute-engine waits. The post-pass below then trims
    # each DMA to <=1 sem wait (walrus's DMA_DIRECT2D sync-wait slot limit).
    _ts.NUM_HWDGE_SEMS = 2
    _tsa.NUM_HWDGE_SEMS = 2

    f32 = mybir.dt.float32
    W = H
    OH = H + 2 * _PAD
    nc = bass.Bass()
    x = nc.dram_tensor("data", (_FACES, C, H, W), f32, kind="ExternalInput")
    y = nc.dram_tensor("out", (_FACES, C, OH, OH), f32, kind="ExternalOutput")

    with TileContext(nc) as tc:
        with (
            tc.tile_pool(name="tiles", bufs=tiles_bufs) as tpool,
            tc.tile_pool(name="planes", bufs=planes_bufs) as ppool,
            tc.tile_pool(name="rows", bufs=4) as rpool,
            tc.tile_pool(name="cols", bufs=26) as cpool,
        ):
            for c0 in range(0, C, PCHUNK):
                P = PCHUNK
                cs = slice(c0, c0 + P)
                toprows = rpool.tile([P, _FACES, 2, W], f32,
                                     name=f"toprows_{c0}", tag="rows")
                botrows = rpool.tile([P, _FACES, 2, W], f32,
                                     name=f"botrows_{c0}", tag="rows")
                nc.sync.dma_start(out=toprows[:],
                                    in_=x[:, cs, 0:2, :].transpose((1, 0, 2, 3)))
                nc.sync.dma_start(out=botrows[:],
                                    in_=x[:, cs, H - 2:H, :].transpose((1, 0, 2, 3)))

                colL, colR, planes = {}, {}, {}
                loaded, assembled = set(), set()
                for f in _ORDER:
                    tile = tpool.tile([P, H, W], f32,
                                      name=f"tile_{c0}_{f}", tag="tile")
                    nc.sync.dma_start(
                        out=tile.rearrange("p a b -> p (a b)"),
                        in_=x[f, cs].rearrange("c a b -> c (a b)"))
                    cl = cpool.tile([P, H, 2], f32, name=f"colL_{c0}_{f}", tag="col")
                    cr = cpool.tile([P, H, 2], f32, name=f"colR_{c0}_{f}", tag="col")
                    nc.vector.tensor_copy(cl[:], tile[:, :, 0:2])
                    nc.vector.tensor_copy(cr[:], tile[:, :, W - 2:W])
                    colL[f], colR[f] = cl, cr
                    pl = ppool.tile([P, OH, OH], f32,
                                    name=f"plane_{c0}_{f}", tag="plane")
                    nc.vector.tensor_copy(pl[:, 2:2 + H, 2:2 + W], tile[:])
                    planes[f] = pl
                    loaded.add(f)
                    for g in _ORDER:
                        if g in assembled or g not in loaded:
                            continue
                        if all(d in loaded for d in _col_deps(g)):
                            _assemble(nc, g, planes[g], colL, colR,
                                      toprows, botrows, H, OH)
                            nc.sync.dma_start(
                                out=y[g, cs].rearrange("c a b -> c (a b)"),
                                in_=planes[g].rearrange("p a b -> p (a b)"))
                            assembled.add(g)
                assert len(assembled) == _FACES

    # walrus's DMA_DIRECT2D lowering accepts a single sync-wait slot, but
    # some DMAs carry two waits (compute sem + DMAHW lane-chain sem). Every
    # DMA here issues from the SP sequencer in program order onto one HWDGE
    # ring (qSPDynamicHW): no two DMAs touch overlapping memory (loads/stores
    # hit disjoint SBUF slots and DRAM face regions), so inter-DMA DMAHW
    # waits are pure ring-capacity bookkeeping. Keep the compute wait when
    # present; otherwise keep the DMA's own-lane chain wait (a throttle that
    # is satisfied ~2 transfers early and costs no bubble).
    import concourse.mybir as mybir
    last_dma = None
    for blk in nc.m.functions[0].blocks:
        for inst in blk.instructions:
            if not isinstance(inst, mybir.InstDMACopy):
                continue
            assert inst.engine == mybir.EngineType.SP, inst.concise()
            last_dma = inst
            si = inst.sync_info
            if si is None:
                continue
            own_lane = None
            for u in si.on_update:
                if u.ant_name.startswith("DMAHW"):
                    own_lane = u.ant_name
            if len(si.on_wait) <= 1:
                continue
            keep = [w for w in si.on_wait if not w.ant_name.startswith("DMAHW")]
            if not keep:
                own = [w for w in si.on_wait if w.ant_name == own_lane]
                keep = [own[0]] if own else [si.on_wait[0]]
            assert len(keep) == 1, [w.ant_name for w in si.on_wait]
            si.on_wait = keep
            inst.sync_info = si

    # The SP kernel-tail Drain waits on [DVE_total, DMAHW0_total,
    # DMAHW1_total]. Ring completion order equals issue order, so the last
    # DMA's own-lane total covers every DMA; that last DMA (the final store)
    # also waits on the final DVE op, so the DVE total is transitively
    # implied. Keep just the last DMA's lane total to fit the 1-wait slot.
    lane_totals = {}
    for blk in nc.m.functions[0].blocks:
        for inst in blk.instructions:
            if isinstance(inst, mybir.InstDMACopy):
                for u in (inst.sync_info.on_update if inst.sync_info else []):
                    if u.ant_name.startswith("DMAHW"):
                        lane_totals[u.ant_name] = (
                            lane_totals.get(u.ant_name, 0) + u.update_value)
    last_lane = None
    for u in last_dma.sync_info.on_update:
        if u.ant_name.startswith("DMAHW"):
            last_lane = u.ant_name
    for blk in nc.m.functions[0].blocks:
        for inst in blk.instructions:
            if not isinstance(inst, mybir.InstDrain):
                continue
            si = inst.sync_info
            if si is None or len(si.on_wait) <= 1:
                continue
            keep = [w for w in si.on_wait if w.ant_name == last_lane]
            assert len(keep) == 1, [w.ant_name for w in si.on_wait]
            assert keep[0].wait_value == lane_totals[last_lane], (
                keep[0].wait_value, lane_totals)
            si.on_wait = keep
            inst.sync_info = si

    nc.finalize()
    return nc


_NC_CACHE = {}


def _get_nc():
    if "nc" not in _NC_CACHE:
        _NC_CACHE["nc"] = _build_nc()
    return _NC_CACHE["nc"]


def _run(data, **kwargs):
    from concourse import bass_utils

    data = np.ascontiguousarray(np.asarray(data, dtype=np.float32))
    n_cores = 8
    group = data.shape[0] // n_cores
    assert group == _FACES
    nc = _get_nc()
    in_maps = [{"data": data[g * group:(g + 1) * group]} for g in range(n_cores)]
    return bass_utils.run_bass_kernel_spmd(
        nc, in_maps, core_ids=list(range(n_cores)), **kwargs)


def kernel(data, p):
    assert int(p) == _PAD
    res = _run(data)
    return np.concatenate([r["out"] for r in res.results], axis=0)



# revision 4
# speedup vs baseline: 1.2093x; 1.0289x over previous
"""HEALPix padding (p=2) kernel v2: byte-floor DMA schedule.

Differences vs v1: no DRAM row staging (rows/cols extracted on-chip from each
face tile), and each face's output is stored as two full-rate DMAs:
  - middle  = output rows 2..65 (per-channel 17408B contiguous run)
  - slabs   = output rows {0,1} and {66,67} merged into ONE DMA via a crafted
              access pattern [(4624,128),(4488,2),(1,136)] (544B runs)
Total DMA bytes = input 50.33MB + output 56.82MB per core — the floor.

The middle of face g needs halo columns from only 2 neighbor faces, so the
load order [1,0,6,10,11,2,7,8,5,9,3,4] lets middles stream out ~2 loads after
their face arrives; slabs (tiny) flush whenever their row deps are in.
"""

import numpy as np

_FACES = 12
_PAD = 2

# Load order: middle deps (2 per face) arrive early; <=4 live middle planes.
_ORDER = [1, 0, 6, 10, 11, 2, 7, 8, 5, 9, 3, 4]


def _mid_deps(g):
    """Faces whose strips feed g's middle left/right halo columns."""
    if g < 4:
        return ((g + 3) % 4, 4 + (g + 1) % 4)
    if g < 8:
        i = g - 4
        return ((i + 3) % 4, 8 + i)
    i = g - 8
    return (4 + i, 8 + (i + 1) % 4)


def _slab_deps(g):
    """Faces whose strips feed g's top/bottom slabs (rows 0:2 and 66:68)."""
    if g < 4:
        i = g
        return {(i + 1) % 4, (i + 2) % 4, 4 + i, (i + 3) % 4, 8 + i}
    if g < 8:
        i = g - 4
        return {i, (i + 3) % 4, 4 + (i + 1) % 4,
                8 + (i + 3) % 4, 8 + i, 4 + (i + 3) % 4}
    i = g - 8
    return {4 + (i + 1) % 4, i, 8 + (i + 1) % 4,
            8 + (i + 3) % 4, 8 + (i + 2) % 4}


def _fill_mid_edges(nc, g, M, topr, botr, colL, colR, H):
    """Left/right 2 halo columns of face g's middle (rows 2..65)."""
    V = nc.vector
    W = H
    if g < 4:  # _pn: left from l's top rows (rot), right from r's first cols
        i = g
        l = (i + 3) % 4
        r = 4 + (i + 1) % 4
        for j in range(2):
            V.tensor_copy(M[:, :, j], topr[l][:, 1 - j, :])
        V.tensor_copy(M[:, :, W + 2:W + 4], colL[r][:])
    elif g < 8:  # _pe: left = colR[l], right = colL[r]
        i = g - 4
        l = (i + 3) % 4
        r = 8 + i
        V.tensor_copy(M[:, :, 0:2], colR[l][:])
        V.tensor_copy(M[:, :, W + 2:W + 4], colL[r][:])
    else:  # _ps: left = colR[l], right from r's bottom rows (rot)
        i = g - 8
        l = 4 + i
        r = 8 + (i + 1) % 4
        V.tensor_copy(M[:, :, 0:2], colR[l][:])
        for j in range(2):
            V.tensor_copy(M[:, :, W + 2 + j], botr[r][:, 1 - j, :])


def _fill_slabs(nc, g, S, topr, botr, colL, colR, H):
    """Assemble top (S[:,0]) and bottom (S[:,1]) 2x68 slabs of face g."""
    V = nc.vector
    W = H
    T = S[:, 0]  # (P, 2, 68) output rows 0:2
    B = S[:, 1]  # (P, 2, 68) output rows 66:68
    Sf = S.rearrange("p s r w -> p (s r w)")
    if g < 4:  # _pn
        i = g
        t = (i + 1) % 4
        tl = (i + 2) % 4
        b = 4 + i
        bl = (i + 3) % 4
        br = 8 + i
        # top rows: rot90 of t's first-2 cols
        for r_ in range(2):
            V.tensor_copy(T[:, r_, 2:2 + W], colL[t][:, :, 1 - r_])
        # tl corner: rot180 of tl's first 2x2
        for i_ in range(2):
            for j_ in range(2):
                V.tensor_copy(T[:, i_:i_ + 1, j_:j_ + 1],
                              topr[tl][:, 1 - i_:2 - i_, 1 - j_:2 - j_])
        V.tensor_copy(T[:, :, W + 2:W + 4], botr[t][:, :, 0:2])  # tr
        V.tensor_copy(B[:, :, 2:2 + W], topr[b][:, :, :])        # bottom
        V.tensor_copy(B[:, :, 0:2], topr[bl][:, :, W - 2:W])     # bl
        V.tensor_copy(B[:, :, W + 2:W + 4], topr[br][:, :, 0:2])  # br
    elif g < 8:  # _pe
        i = g - 4
        t = i
        l = (i + 3) % 4
        tr = 4 + (i + 1) % 4
        b = 8 + (i + 3) % 4
        r = 8 + i
        bl = 4 + (i + 3) % 4
        V.tensor_copy(T[:, :, 2:2 + W], botr[t][:, :, :])        # top
        V.tensor_copy(B[:, :, 2:2 + W], topr[b][:, :, :])        # bottom
        # tl corner (computed): diag = .5*(t[H-2:H,0] + l[0,W-2:W])
        V.tensor_copy(T[:, 0:1, 1:2], colL[t][:, H - 2:H - 1, 0:1])
        V.tensor_copy(T[:, 1:2, 0:1], topr[l][:, 0:1, W - 2:W - 1])
        d = Sf[:, 0:70:69]
        V.tensor_add(d, colL[t].rearrange("p a b -> p (a b)")[:, 2 * (H - 2):2 * H:2],
                     topr[l].rearrange("p a b -> p (a b)")[:, W - 2:W])
        V.tensor_scalar_mul(d, d, 0.5)
        # br corner (computed): diag = .5*(b[0:2,W-1] + r[H-1,0:2])
        V.tensor_copy(B[:, 0:1, W + 3:W + 4], botr[r][:, 1:2, 1:2])
        V.tensor_copy(B[:, 1:2, W + 2:W + 3], topr[b][:, 1:2, W - 1:W])
        d = Sf[:, 2 * 68 + 66:2 * 68 + 66 + 70:69]
        V.tensor_add(d, topr[b].rearrange("p a b -> p (a b)")[:, W - 1:2 * W:W],
                     botr[r].rearrange("p a b -> p (a b)")[:, W:W + 2])
        V.tensor_scalar_mul(d, d, 0.5)
        V.tensor_copy(B[:, :, 0:2], topr[bl][:, :, W - 2:W])     # bl
        V.tensor_copy(T[:, :, W + 2:W + 4], botr[tr][:, :, 0:2])  # tr
    else:  # _ps
        i = g - 8
        t = 4 + (i + 1) % 4
        tl = i
        tr = 8 + (i + 1) % 4
        b = 8 + (i + 3) % 4
        bl = b
        br = 8 + (i + 2) % 4
        V.tensor_copy(T[:, :, 2:2 + W], botr[t][:, :, :])        # top
        V.tensor_copy(T[:, :, 0:2], botr[tl][:, :, W - 2:W])     # tl
        V.tensor_copy(T[:, :, W + 2:W + 4], botr[tr][:, :, 0:2])  # tr
        # bottom rows: rot90 of b's last-2 cols
        for r_ in range(2):
            V.tensor_copy(B[:, r_, 2:2 + W], colR[b][:, :, 1 - r_])
        V.tensor_copy(B[:, :, 0:2], topr[bl][:, :, W - 2:W])     # bl
        # br corner: rot180 of br's last 2x2
        for i_ in range(2):
            for j_ in range(2):
                V.tensor_copy(B[:, i_:i_ + 1, W + 2 + j_:W + 3 + j_],
                              botr[br][:, 1 - i_:2 - i_, W - 1 - j_:W - j_])


def _slab_out_ap(y, g, cs, H):
    """Crafted DRAM AP for rows {0,1,66,67} of y[g, cs]: (128, 2, 136)."""
    OH = H + 4
    ap = y[g, cs].copy()
    v = ap.ap
    v.clear()
    v.extend([(OH * OH, 128), ((OH - 2) * OH, 2), (1, 2 * OH)])
    return ap


def _build_nc(C=256, H=64, PCHUNK=128):
    import concourse.bass as bass
    import concourse.mybir as mybir
    import concourse.tile_scheduler as _ts
    import concourse.tile_sem_assignment as _tsa
    from concourse.tile import TileContext

    # Eight completion-tracking lanes: with one lane the framework chains
    # every DMA on its predecessor's full completion (2175ns bubble each).
    # With eight, the same-lane chain wait is "DMA k-8 complete" — satisfied
    # long before issue even across clusters of tiny slab stores.
    _ts.NUM_HWDGE_SEMS = 8
    _tsa.NUM_HWDGE_SEMS = 8

    f32 = mybir.dt.float32
    W = H
    OH = H + 2 * _PAD
    nc = bass.Bass()
    x = nc.dram_tensor("data", (_FACES, C, H, W), f32, kind="ExternalInput")
    y = nc.dram_tensor("out", (_FACES, C, OH, OH), f32, kind="ExternalOutput")

    with TileContext(nc) as tc:
        with (
            tc.tile_pool(name="tiles", bufs=3) as tpool,
            tc.tile_pool(name="mids", bufs=6) as mpool,
            tc.tile_pool(name="strips", bufs=56) as spool,
            tc.tile_pool(name="slabs", bufs=14) as bpool,
        ):
            for c0 in range(0, C, PCHUNK):
                P = PCHUNK
                cs = slice(c0, c0 + P)
                topr, botr, colL, colR = {}, {}, {}, {}
                mids, slabs = {}, {}
                loaded = set()
                mid_done, slab_done = set(), set()
                store_q = []  # (kind, g) queued last step, emit this step

                def emit_store(kind, g):
                    if kind == "mid":
                        nc.sync.dma_start(
                            out=y[g, cs, 2:2 + H, :].rearrange("c a b -> c (a b)"),
                            in_=mids[g].rearrange("p a b -> p (a b)"))
                    else:
                        nc.sync.dma_start(
                            out=_slab_out_ap(y, g, cs, H),
                            in_=slabs[g].rearrange("p s r w -> p (s r) w"))

                for step, f in enumerate(_ORDER):
                    tile = tpool.tile([P, H, W], f32, name=f"tile_{c0}_{f}",
                                      tag="tile")
                    nc.sync.dma_start(
                        out=tile.rearrange("p a b -> p (a b)"),
                        in_=x[f, cs].rearrange("c a b -> c (a b)"))
                    # Emit stores whose DVE assembly was queued last step —
                    # their waits are long satisfied; ring never stalls.
                    for kind, g in store_q:
                        emit_store(kind, g)
                    store_q = []
                    # Extract this face's strips (first for dependents).
                    tr_ = spool.tile([P, 2, W], f32, name=f"topr_{c0}_{f}", tag="s")
                    br_ = spool.tile([P, 2, W], f32, name=f"botr_{c0}_{f}", tag="s")
                    cl = spool.tile([P, H, 2], f32, name=f"colL_{c0}_{f}", tag="s")
                    cr = spool.tile([P, H, 2], f32, name=f"colR_{c0}_{f}", tag="s")
                    nc.vector.tensor_copy(tr_[:], tile[:, 0:2, :])
                    nc.vector.tensor_copy(br_[:], tile[:, H - 2:H, :])
                    nc.vector.tensor_copy(cl[:], tile[:, :, 0:2])
                    nc.vector.tensor_copy(cr[:], tile[:, :, W - 2:W])
                    topr[f], botr[f], colL[f], colR[f] = tr_, br_, cl, cr
                    loaded.add(f)
                    # Assemble newly-ready middles of OTHER faces (their
                    # interiors are done; only edges from f's strips needed).
                    for g in _ORDER:
                        if (g in mid_done or g == f or g not in loaded
                                or not all(d in loaded for d in _mid_deps(g))):
                            continue
                        _fill_mid_edges(nc, g, mids[g], topr, botr, colL, colR, H)
                        store_q.append(("mid", g))
                        mid_done.add(g)
                    # This face's interior (bulk DVE work last).
                    M = mpool.tile([P, H, OH], f32, name=f"mid_{c0}_{f}", tag="mid")
                    nc.vector.tensor_copy(M[:, :, 2:2 + W], tile[:])
                    mids[f] = M
                    # f's own middle, if its deps already arrived.
                    if all(d in loaded for d in _mid_deps(f)):
                        _fill_mid_edges(nc, f, M, topr, botr, colL, colR, H)
                        store_q.append(("mid", f))
                        mid_done.add(f)
                    # Newly-ready slabs.
                    for g in _ORDER:
                        if (g in slab_done or g not in loaded
                                or not _slab_deps(g) <= loaded):
                            continue
                        S = bpool.tile([P, 2, 2, OH], f32,
                                       name=f"slab_{c0}_{g}", tag="slab")
                        _fill_slabs(nc, g, S, topr, botr, colL, colR, H)
                        slabs[g] = S
                        store_q.append(("slab", g))
                        slab_done.add(g)
                # Flush remaining queued stores, slabs first: the kernel-tail
                # Drain waits only on the final DMA's lane total, so the final
                # DMA should be a big middle store (completes last on HW).
                for kind, g in sorted(store_q, key=lambda t: t[0] == "mid"):
                    emit_store(kind, g)
                assert len(mid_done) == _FACES and len(slab_done) == _FACES

    # walrus accepts a single sync-wait slot per instruction, so reduce every
    # multi-wait instruction to one wait. All DMAs issue from the SP sequencer
    # onto one HWDGE ring (qSPDynamicHW) whose full-completion order equals
    # issue order (the invariant the framework's own cross-lane wait pruning
    # and the baseline kernel already rely on), so "lane L count m complete"
    # is a statement about a ring position, comparable across lanes.
    #
    # Pass 1 — compute instructions with [DVE>=t, DMAHW_L>=v]: the DVE wait is
    # a real DVE-pipeline hazard; the DMAHW wait is implied whenever the
    # ring-order closure of what DVE ops ticking <=t waited on reaches v's
    # ring position. Drop implied DMAHW waits (multi-hop transitivity the
    # framework's one-hop pruner misses).
    import concourse.mybir as mybir

    ring_pos = {}  # (lane_name, count_on_lane) -> ring index
    lane_counts = {}
    for blk in nc.m.functions[0].blocks:
        for inst in blk.instructions:
            if not isinstance(inst, mybir.InstDMACopy):
                continue
            for u in (inst.sync_info.on_update if inst.sync_info else []):
                if u.ant_name.startswith("DMAHW"):
                    lane_counts[u.ant_name] = (
                        lane_counts.get(u.ant_name, 0) + u.update_value)
                    ring_pos[(u.ant_name, lane_counts[u.ant_name])] = (
                        len(ring_pos))

    def wait_ring_pos(w):
        # wait >= v is satisfied when this lane's count first reaches >= v;
        # lane counts move in +16 steps, so round v up to the next multiple.
        v = -(-w.wait_value // 16) * 16
        assert (w.ant_name, v) in ring_pos, (w.ant_name, w.wait_value)
        return ring_pos[(w.ant_name, v)]

    know = [-1]  # know[t] = max ring index implied once DVE tick >= t
    for blk in nc.m.functions[0].blocks:
        for inst in blk.instructions:
            si = inst.sync_info
            if si is None:
                continue
            dve_waits = [w for w in si.on_wait if w.ant_name.startswith("DVE")]
            hw_waits = [w for w in si.on_wait if w.ant_name.startswith("DMAHW")]
            if (dve_waits and hw_waits
                    and not isinstance(inst, (mybir.InstDMACopy,
                                              mybir.InstDrain))):
                assert len(dve_waits) == 1
                t = min(dve_waits[0].wait_value, len(know) - 1)
                keep_hw = [w for w in hw_waits if wait_ring_pos(w) > know[t]]
                assert not keep_hw, (
                    "DMAHW wait not implied by DVE wait on " + inst.concise())
                si.on_wait = [w for w in si.on_wait
                              if not w.ant_name.startswith("DMAHW")]
                inst.sync_info = si
                hw_waits = []
            if inst.engine == mybir.EngineType.DVE:
                ticks = sum(u.update_value for u in si.on_update
                            if u.ant_name.startswith("DVE"))
                if ticks:
                    cur = know[-1]
                    for w in hw_waits:
                        cur = max(cur, wait_ring_pos(w))
                    know.extend([cur] * ticks)

    # Pass 2 — trim every DMA to <=1 sem wait: keep the compute wait when
    # present, else the DMA's own-lane chain wait.
    last_dma = None
    for blk in nc.m.functions[0].blocks:
        for inst in blk.instructions:
            if not isinstance(inst, mybir.InstDMACopy):
                continue
            assert inst.engine == mybir.EngineType.SP, inst.concise()
            last_dma = inst
            si = inst.sync_info
            if si is None:
                continue
            own_lane = None
            for u in si.on_update:
                if u.ant_name.startswith("DMAHW"):
                    own_lane = u.ant_name
            if len(si.on_wait) <= 1:
                continue
            keep = [w for w in si.on_wait if not w.ant_name.startswith("DMAHW")]
            if not keep:
                own = [w for w in si.on_wait if w.ant_name == own_lane]
                keep = [own[0]] if own else [si.on_wait[0]]
            assert len(keep) == 1, [w.ant_name for w in si.on_wait]
            si.on_wait = keep
            inst.sync_info = si

    # Drain: ring completion order == issue order, so the last DMA's own-lane
    # total covers every DMA; and since some DMA (a flushed store) waits on
    # the final DVE tick, "all DMAs complete" also implies "all DVE done".
    # Keep just the last DMA's lane-total wait to fit the slot.
    lane_totals = {}
    max_dve_wait_on_dma = 0
    for blk in nc.m.functions[0].blocks:
        for inst in blk.instructions:
            if isinstance(inst, mybir.InstDMACopy):
                si = inst.sync_info
                for w in (si.on_wait if si else []):
                    if w.ant_name.startswith("DVE"):
                        max_dve_wait_on_dma = max(max_dve_wait_on_dma,
                                                  w.wait_value)
                for u in (si.on_update if si else []):
                    if u.ant_name.startswith("DMAHW"):
                        lane_totals[u.ant_name] = (
                            lane_totals.get(u.ant_name, 0) + u.update_value)
    last_lane = None
    for u in last_dma.sync_info.on_update:
        if u.ant_name.startswith("DMAHW"):
            last_lane = u.ant_name
    for blk in nc.m.functions[0].blocks:
        for inst in blk.instructions:
            if not isinstance(inst, mybir.InstDrain):
                continue
            si = inst.sync_info
            if si is None or len(si.on_wait) <= 1:
                continue
            dve = [w for w in si.on_wait if w.ant_name.startswith("DVE")]
            assert all(w.wait_value <= max_dve_wait_on_dma for w in dve), (
                "drain DVE wait not covered by any DMA's DVE wait")
            keep = [w for w in si.on_wait if w.ant_name == last_lane]
            assert len(keep) == 1, [w.ant_name for w in si.on_wait]
            assert keep[0].wait_value == lane_totals[last_lane], (
                keep[0].wait_value, lane_totals)
            si.on_wait = keep
            inst.sync_info = si

    nc.finalize()
    return nc


_NC_CACHE = {}


def _get_nc():
    if "nc" not in _NC_CACHE:
        _NC_CACHE["nc"] = _build_nc()
    return _NC_CACHE["nc"]


def _run(data, **kwargs):
    from concourse import bass_utils

    data = np.ascontiguousarray(np.asarray(data, dtype=np.float32))
    n_cores = 8
    group = data.shape[0] // n_cores
    assert group == _FACES
    nc = _get_nc()
    in_maps = [{"data": data[g * group:(g + 1) * group]} for g in range(n_cores)]
    return bass_utils.run_bass_kernel_spmd(
        nc, in_maps, core_ids=list(range(n_cores)), **kwargs)


def kernel(data, p):
    assert int(p) == _PAD
    res = _run(data)
    return np.concatenate([r["out"] for r in res.results], axis=0)
